# revision 68
# baseline (speedup 1.0000x reference)
# Trainium2 Bass kernel for the ASE (axial squeeze attention) block.
#
# Sharding: pure data parallel over batch B=16 across 8 NeuronCores
# (2 batches per core); all params replicated.
#
# Math restructuring (host-side folds):
#  - BN scales fold into conv weights; biases applied during PSUM evictions.
#  - 1x1 convs commute with spatial means, so row/col attention only needs
#    the row/col means of x (256x64 each), never full q/k maps.
#  - positional embeddings interpolated on host and folded into qr/kr biases.
#  - softmax: exp on ACT; denominator via ones-matmul; normalization by
#    broadcasting the reciprocal row with a K=1 matmul.
#  - h_sigmoid(x)*gate = min(relu(x+3), 6) * (gate/6): the /6 folds into the
#    pointwise conv weights; min*mult fused in one scalar_tensor_tensor.
import numpy as np

import concourse.bass as bass
import concourse.mybir as mybir
import concourse.tile as tile
from concourse import bacc, bass_utils

F32 = mybir.dt.float32
F32R = mybir.dt.float32r
BF16 = mybir.dt.bfloat16
AF = mybir.ActivationFunctionType
ALU = mybir.AluOpType
AX = mybir.AxisListType

B, DIM, H, W = 16, 256, 64, 64
KEY_DIM, HEADS = 16, 8
NH_KD = KEY_DIM * HEADS   # 128
DH = 2 * KEY_DIM * HEADS  # 256
POS = 16
N_CORES = 8
BPC = B // N_CORES        # batches per core

MMDT = F32R  # dtype of every tensor feeding the PE (set by build_nc)


def _r(ap):
    return ap


def build_nc(bpc=BPC, h=H, w=W, chunk_h=8, num_devices=N_CORES, use_f32r=True,
             nrep=1, tiny_out=False):
    """Build the per-core Bass module. Parameterized so a mini config can be
    simulated in CoreSim."""
    global MMDT
    MMDT = F32R if use_f32r else F32
    hw = h * w
    n_chunks = h // chunk_h
    nc_cols = chunk_h * w      # spatial columns per chunk

    nc = bacc.Bacc("TRN2", target_bir_lowering=False, debug=False,
                   num_devices=num_devices)

    dram = {}

    def din(name, shape, dt=None):
        dram[name] = nc.dram_tensor(name, shape, dt or F32,
                                    kind="ExternalInput").ap()
        return dram[name]

    x_d = din("x", (bpc, DIM, hw), MMDT)
    wbig_d = din("wbig", (128, 2688), MMDT)    # packed f32r weights
    wbigb_d = din("wbigb", (128, 2624), BF16)  # packed bf16 attention weights
    qkb_d = din("qkbias", (32, 2 * HEADS * (h + w)))
    params_d = din("params", (128, 20))
    onesw_d = din("onesW", (max(h, w), 1), BF16)
    ones1_d = din("ones1", (1, 128), BF16)
    y_cols = nc_cols if tiny_out else hw
    y_d = nc.dram_tensor("y", (bpc, DIM, y_cols), F32,
                         kind="ExternalOutput").ap()

    with tile.TileContext(nc) as tc:
        _emit(nc, tc, dram, y_d, bpc, h, w, hw, chunk_h, n_chunks, nc_cols,
              nrep, tiny_out)
    nc.compile()
    return nc


def _emit(nc, tc, dram, y_d, bpc, h, w, hw, chunk_h, n_chunks, nc_cols,
          nrep=1, tiny_out=False):
    from contextlib import ExitStack
    with ExitStack() as _ctx:
        _emit_body(_ctx, nc, tc, dram, y_d, bpc, h, w, hw, chunk_h, n_chunks,
                   nc_cols, nrep, tiny_out)


def _emit_body(ctx, nc, tc, dram, y_d, bpc, h, w, hw, chunk_h, n_chunks,
               nc_cols, nrep=1, tiny_out=False):
    ctx.enter_context(nc.allow_low_precision(
        reason="fp32r matmul operand rounding"))
    # ---- persistent weights / params (loaded once) ----
    wp = ctx.enter_context(tc.tile_pool(name="weights", bufs=1))

    def load2(dap, cols, dt=None):
        # (256, cols) DRAM -> two (128, cols) SBUF k-tiles
        ts = []
        for k in range(2):
            t = wp.tile([128, cols], dt or MMDT,
                        tag=f"w_{dap.tensor.name}_{k}")
            nc.sync.dma_start(out=t, in_=dap[128 * k:128 * (k + 1), :])
            ts.append(t)
        return ts

    # packed weight loads: one big contiguous DMA each
    wbig = wp.tile([128, 2688], MMDT, tag="wbig")
    nc.sync.dma_start(out=wbig, in_=dram["wbig"])
    wbigb = wp.tile([128, 2624], BF16, tag="wbigb")
    nc.sync.dma_start(out=wbigb, in_=dram["wbigb"])
    params = wp.tile([128, 20], F32, tag="params")
    nc.sync.dma_start(out=params, in_=dram["params"])
    qkb = wp.tile([32, 2 * HEADS * (h + w)], F32, tag="qkb")
    nc.sync.dma_start(out=qkb, in_=dram["qkbias"])

    def _slices(tile_, widths):
        out, off = [], 0
        for wd in widths:
            out.append(tile_[:, off:off + wd])
            off += wd
        return out

    (wqT0, wqT1, wkT0, wkT1, wvT0, wvT1, wpw0, wpw1, wpw2, wpw3,
     wpT0, wpT1, ident128) = _slices(
        wbig, [NH_KD, NH_KD, NH_KD, NH_KD, DH, DH, DIM, DIM, DIM, DIM,
               DIM, DIM, 128])
    wqT, wkT, wvT = [wqT0, wqT1], [wkT0, wkT1], [wvT0, wvT1]
    wpwT, wpT = [wpw0, wpw1, wpw2, wpw3], [wpT0, wpT1]
    (wrT0, wrT1, wcT0, wcT1, wqTp0, wqTp1, wkTp0, wkTp1, wvTb0, wvTb1,
     ident64b) = _slices(
        wbigb, [DH, DH, DH, DH, 256, 256, 256, 256, DH, DH, 64])
    wrT, wcT = [wrT0, wrT1], [wcT0, wcT1]
    wqTp, wkTp, wvTb = [wqTp0, wqTp1], [wkTp0, wkTp1], [wvTb0, wvTb1]
    ident64 = ident64b[:64, :]
    qr_bias, kr_bias, qc_bias, kc_bias = _slices(
        qkb, [HEADS * h, HEADS * h, HEADS * w, HEADS * w])
    onesW = wp.tile([max(h, w), 1], BF16, tag="onesW")   # value = W (mean fold)
    nc.sync.dma_start(out=onesW, in_=dram["onesW"])
    ones1 = wp.tile([1, 128], BF16, tag="ones1")
    nc.sync.dma_start(out=ones1, in_=dram["ones1"])

    # param columns
    zscale = [params[:, g:g + 1] for g in range(4)]
    zbias = [params[:, 4 + g:5 + g] for g in range(4)]
    bv_att = [params[:, 8 + m:9 + m] for m in range(2)]
    brv = [params[:, 10 + m:11 + m] for m in range(2)]
    bcc = [params[:, 12 + m:13 + m] for m in range(2)]
    bp3 = [params[:, 14 + m:15 + m] for m in range(2)]
    bpw6 = [params[:, 16 + m:17 + m] for m in range(2)]

    # ---- pools ----
    px = ctx.enter_context(tc.tile_pool(name="x", bufs=2))
    pa = ctx.enter_context(tc.tile_pool(name="attn", bufs=1))
    pz = ctx.enter_context(tc.tile_pool(name="z", bufs=5))
    pc = ctx.enter_context(tc.tile_pool(name="chunk", bufs=3))
    pout = ctx.enter_context(tc.tile_pool(name="outb", bufs=3))
    pp = ctx.enter_context(tc.tile_pool(name="psum", bufs=1, space="PSUM"))

    def attention(xm, nseq, q_bias, k_bias, wconvT, bconv, tagp):
        """Axial attention along one axis.
        xm: [2](128, nseq) column-sums of x  (sum, not mean; folds handled)
        returns xatt: [2](128, nseq) tiles (conv output + bias, pre-broadcast).
        """
        # qr/kr: per-head M=32 matmuls into free slices (partitions 0-31,
        # 16 real kd channels + 16 zero pad), then one bias add per tensor
        qk_all = []
        for wi, (wT, bias_t) in enumerate(((wqTp, q_bias), (wkTp, k_bias))):
            ps = pp.tile([32, HEADS * nseq], F32, tag="att",
                         name=f"at_qk_{tagp}_{wi}")
            for hh in range(HEADS):
                for k in range(2):
                    nc.tensor.matmul(ps[:, nseq * hh:nseq * (hh + 1)],
                                     lhsT=_r(wT[k][:, 32 * hh:32 * (hh + 1)]),
                                     rhs=_r(xm[k]),
                                     start=(k == 0), stop=(k == 1))
            sb = pa.tile([32, HEADS * nseq], BF16, tag=f"at_qks_{tagp}_{wi}",
                         name=f"at_qks_{tagp}_{wi}")
            nc.vector.tensor_tensor(out=sb, in0=ps, in1=bias_t, op=ALU.add)
            qk_all.append(sb)
        qr_all, kr_all = qk_all
        # vrT (nseq, 256) = xm.T @ Wv.T
        vrT_ps = pp.tile([nseq, DH], F32, tag="att", name="vrT_ps")
        for k in range(2):
            nc.tensor.matmul(vrT_ps, lhsT=_r(xm[k]), rhs=_r(wvTb[k]),
                             start=(k == 0), stop=(k == 1))
        vrT = pa.tile([nseq, DH], BF16, tag=f"at_vs_{tagp}")
        nc.vector.tensor_copy(out=vrT, in_=vrT_ps)
        # scoresT: (nseq_j, 8*nseq_i); all K=32 matmuls at partition 0
        st_ps = pp.tile([nseq, HEADS * nseq], F32, tag="att", name="st_ps")
        for hh in range(HEADS):
            sl = slice(nseq * hh, nseq * (hh + 1))
            nc.tensor.matmul(st_ps[:, sl], lhsT=_r(kr_all[:, sl]),
                             rhs=_r(qr_all[:, sl]), start=True, stop=True)
        eT = pa.tile([nseq, HEADS * nseq], BF16, tag=f"at_e_{tagp}")
        nc.scalar.activation(out=eT, in_=st_ps, func=AF.Exp,
                             scale=KEY_DIM ** -0.5)
        # denominator row (scaled by W via onesW value) and reciprocal
        srow_ps = pp.tile([1, HEADS * nseq], F32, tag="att", name="srow_ps")
        nc.tensor.matmul(srow_ps, lhsT=_r(onesW[:nseq, :]), rhs=_r(eT),
                         start=True, stop=True)
        recip = pa.tile([1, HEADS * nseq], BF16, tag=f"at_rc_{tagp}")
        nc.vector.reciprocal(out=recip, in_=srow_ps)
        # broadcast recip across partitions with a K=1 matmul, then normalize
        rb_ps = pp.tile([128, HEADS * nseq], F32, tag="att", name="rb_ps")
        nc.tensor.matmul(rb_ps, lhsT=_r(ones1), rhs=_r(recip),
                         start=True, stop=True)
        eTn = pa.tile([nseq, HEADS * nseq], BF16, tag=f"at_en_{tagp}")
        nc.vector.tensor_tensor(out=eTn, in0=eT, in1=rb_ps[:nseq, :], op=ALU.mult)
        # attention output, head-major partitions: (256, nseq)
        # attention out, transposed: xrT[i, 32h+d] (dst partitions always 0-63)
        xrT_ps = pp.tile([nseq, DH], F32, tag="att", name="xrT_ps")
        for hh in range(HEADS):
            nc.tensor.matmul(xrT_ps[:, 32 * hh:32 * (hh + 1)],
                             lhsT=_r(eTn[:, nseq * hh:nseq * (hh + 1)]),
                             rhs=_r(vrT[:, 32 * hh:32 * (hh + 1)]),
                             start=True, stop=True)
        xrT_sb = pa.tile([nseq, DH], BF16, tag=f"at_xt_{tagp}")
        nc.vector.tensor_copy(out=xrT_sb, in_=xrT_ps)
        # transpose back to (channel, i) and relu(+bv) on eviction
        xr_relu = []
        for t in range(2):
            tr_ps = pp.tile([128, nseq], BF16, tag="att",
                            name=f"at_tr_{tagp}_{t}")
            nc.tensor.transpose(tr_ps, _r(xrT_sb[:, 128 * t:128 * (t + 1)]),
                                _r(ident64[:nseq, :nseq]))
            sb = pa.tile([128, nseq], BF16, tag=f"at_xrr_{tagp}_{t}",
                         name=f"at_xrr_{tagp}_{t}")
            nc.scalar.activation(out=sb, in_=tr_ps, func=AF.Relu,
                                 bias=bv_att[t])
            xr_relu.append(sb)
        # conv (dh -> dh) + bias
        xatt = []
        for m in range(2):
            ps = pp.tile([128, nseq], F32, tag="att",
                         name=f"at_cv_{tagp}_{m}")
            for k in range(2):
                nc.tensor.matmul(ps,
                                 lhsT=_r(wconvT[k][:, 128 * m:128 * (m + 1)]),
                                 rhs=_r(xr_relu[k]),
                                 start=(k == 0), stop=(k == 1))
            sb = pa.tile([128, nseq], F32, tag=f"at_xa_{tagp}_{m}",
                         name=f"at_xa_{tagp}_{m}")
            nc.scalar.activation(out=sb, in_=ps, func=AF.Identity,
                                 bias=bconv[m])
            xatt.append(sb)
        return xatt

    def phase_load(b):
        xs = []
        for k in range(2):
            t = px.tile([128, hw], MMDT, tag=f"xs{k}")
            for c in range(n_chunks):
                cs0 = slice(c * nc_cols, (c + 1) * nc_cols)
                nc.sync.dma_start(out=t[:, cs0],
                                  in_=dram["x"][b, 128 * k:128 * (k + 1), cs0])
            xs.append(t)
        return xs

    def phase_means(xs):
        # means via identity-matmul accumulation on PE (frees the DVE):
        #  W-dir: 8 accumulating matmuls over w-groups -> psum[c,(h,t)],
        #  then a small DVE reduce over t. H-dir analogous over h-chunks.
        xmW, xmH = [], []
        wt = max(w // 8, 1)
        wgroups = w // wt
        with nc.allow_low_precision(reason="f32r matmul operands"):
            for k in range(2):
                psw = pp.tile([128, h * wt], F32, tag=f"mm{k}", name="ps_meanw")
                xv = xs[k].rearrange("p (h j t) -> p j h t", j=wgroups, t=wt)
                for j in range(wgroups):
                    nc.tensor.matmul(psw, lhsT=ident128, rhs=xv[:, j],
                                     start=(j == 0), stop=(j == wgroups - 1))
                mw = pa.tile([128, h], BF16, tag="xmW", bufs=4)
                nc.vector.tensor_reduce(
                    out=mw.unsqueeze(-1),
                    in_=psw.rearrange("p (h t) -> p h t", t=wt),
                    axis=AX.X, op=ALU.add)
                xmW.append(mw)
                psh = pp.tile([128, nc_cols], F32, tag=f"mm{2 + k}", name="ps_meanh")
                for c in range(n_chunks):
                    nc.tensor.matmul(
                        psh, lhsT=ident128,
                        rhs=xs[k][:, c * nc_cols:(c + 1) * nc_cols],
                        start=(c == 0), stop=(c == n_chunks - 1))
                mh = pa.tile([128, w], BF16, tag="xmH", bufs=4)
                nc.vector.tensor_reduce(
                    out=mh.unsqueeze(-1),
                    in_=psh.rearrange("p (s w) -> p w s", w=w),
                    axis=AX.X, op=ALU.add)
                xmH.append(mh)

        return xmW, xmH

    def phase_attn(xmW, xmH):
        xr_f = attention(xmW, h, qr_bias, kr_bias, wrT, brv, "r")
        xc_f = attention(xmH, w, qc_bias, kc_bias, wcT, bcc, "c")
        return xr_f, xc_f

    def phase_chunks(b, xs, xr_f, xc_f, c_lo=0, c_hi=None):
        for c in range(c_lo, c_hi if c_hi is not None else n_chunks):
            cs = slice(c * nc_cols, (c + 1) * nc_cols)
            hs = slice(c * chunk_h, (c + 1) * chunk_h)
            # q/k/v matmuls
            grp_ps = []
            for gi, (wT, mo) in enumerate(((wqT, 0), (wkT, 0),
                                           (wvT, 0), (wvT, 1))):
                ps = pp.tile([128, nc_cols], F32, tag=f"mm{gi}")
                for k in range(2):
                    nc.tensor.matmul(
                        ps, lhsT=_r(wT[k][:, 128 * mo:128 * (mo + 1)]),
                        rhs=_r(xs[k][:, cs]), start=(k == 0), stop=(k == 1))
                grp_ps.append(ps)
            # gated z eviction (ACT: relu(g*qkv + b))
            z = []
            for g in range(4):
                sb = pz.tile([128, nc_cols], MMDT, tag=f"z{g}")
                nc.scalar.activation(out=sb, in_=grp_ps[g], func=AF.Relu,
                                     scale=zscale[g], bias=zbias[g])
                z.append(sb)
            # xx = relu(v + xr + xc): rc on Pool, add on DVE, relu on Pool
            xx = []
            for m in range(2):
                rc = pc.tile([128, chunk_h, w], F32, tag=f"rc{m}")
                nc.gpsimd.tensor_tensor(
                    out=rc,
                    in0=xr_f[m][:, hs].unsqueeze(-1).broadcast_to(
                        (128, chunk_h, w)),
                    in1=xc_f[m].unsqueeze(1).broadcast_to((128, chunk_h, w)),
                    op=ALU.add)
                xp_sb = pc.tile([128, nc_cols], F32, tag=f"xxp{m}",
                                name=f"xxp{m}")
                nc.vector.scalar_tensor_tensor(
                    out=xp_sb, in0=grp_ps[2 + m], scalar=1.0,
                    in1=rc.rearrange("p h w -> p (h w)"),
                    op0=ALU.mult, op1=ALU.add)
                sb = pc.tile([128, nc_cols], MMDT, tag=f"xx{m}")
                nc.gpsimd.tensor_scalar_max(out=sb, in0=xp_sb, scalar1=0.0)
                xx.append(sb)
            # pointwise conv (512 -> 256); bias added in-place on ACT
            qkv_ps = []
            for m in range(2):
                ps = pp.tile([128, nc_cols], F32, tag=f"o{m}")
                for k in range(4):
                    nc.tensor.matmul(
                        ps, lhsT=_r(wpwT[k][:, 128 * m:128 * (m + 1)]),
                        rhs=_r(z[k]), start=(k == 0), stop=(k == 3))
                nc.scalar.activation(out=ps, in_=ps, func=AF.Identity,
                                     bias=bpw6[m])
                qkv_ps.append(ps)
            # final conv (256 -> 256), relu+bias on DVE, gate stt on DVE
            for m in range(2):
                ps = pp.tile([128, nc_cols], F32, tag="p0", name=f"xp{m}")
                for k in range(2):
                    nc.tensor.matmul(
                        ps, lhsT=_r(wpT[k][:, 128 * m:128 * (m + 1)]),
                        rhs=_r(xx[k]), start=(k == 0), stop=(k == 1))
                r = pc.tile([128, nc_cols], F32, tag=f"r{m}")
                nc.vector.tensor_scalar(out=r, in0=ps, scalar1=bp3[m],
                                        scalar2=0.0, op0=ALU.add, op1=ALU.max)
                o = pout.tile([128, nc_cols], F32, tag=f"ob{m}")
                nc.vector.scalar_tensor_tensor(
                    out=o, in0=r, scalar=6.0, in1=qkv_ps[m],
                    op0=ALU.min, op1=ALU.mult)
                ocs = slice(0, nc_cols) if tiny_out else cs
                nc.sync.dma_start(out=y_d[b, 128 * m:128 * (m + 1), ocs],
                                  in_=o)

    for _ in range(nrep):
        # Emission order = engine FIFO + tag-grant order. Chunks lead (they
        # only need x); means+attention for the NEXT batch are emitted half
        # way through the previous batch's chunks so their results are ready
        # without a pipeline bubble.
        held = None
        half = n_chunks // 2
        for b in range(bpc):
            xs = phase_load(b)
            if held is not None:
                phase_chunks(*held, c_lo=0, c_hi=half)
            xmW, xmH = phase_means(xs)
            at = phase_attn(xmW, xmH)
            if held is not None:
                phase_chunks(*held, c_lo=half)
            held = (b, xs, *at)
        phase_chunks(*held)


# ---------------------------------------------------------------------------
# host-side preparation
# ---------------------------------------------------------------------------

def _interp_pos_np(pe, n):
    s = pe.shape[-1]
    pos = np.clip((np.arange(n, dtype=np.float64) + 0.5) * (s / n) - 0.5,
                  0.0, s - 1.0).astype(np.float32)
    i0 = np.floor(pos).astype(np.int32)
    i1 = np.minimum(i0 + 1, s - 1)
    fw = (pos - i0).astype(np.float32)
    return pe[:, i0] * (1.0 - fw) + pe[:, i1] * fw


def prepare_consts(inputs, h=H, w=W, chunk_h=8):
    """Fold BN/scales and build the constant tensors the kernel expects."""
    import ml_dtypes
    f = lambda a: np.ascontiguousarray(np.asarray(a, dtype=np.float32))
    fb = lambda a: np.ascontiguousarray(
        np.asarray(a, dtype=np.float32).astype(ml_dtypes.bfloat16))
    Wq, sq, bq = f(inputs["Wq"]), f(inputs["sq"]), f(inputs["bq"])
    Wk, sk, bk = f(inputs["Wk"]), f(inputs["sk"]), f(inputs["bk"])
    Wv, sv, bv = f(inputs["Wv"]), f(inputs["sv"]), f(inputs["bv"])
    wdw, sdw, bdw = f(inputs["wdw"]), f(inputs["sdw"]), f(inputs["bdw"])
    Wpw, spw, bpw = f(inputs["Wpw"]), f(inputs["spw"]), f(inputs["bpw"])
    Wr, sr, br = f(inputs["Wr"]), f(inputs["sr"]), f(inputs["br"])
    Wc, sc, bc = f(inputs["Wc"]), f(inputs["sc"]), f(inputs["bc"])
    Wp, sp, bp = f(inputs["Wp"]), f(inputs["sp"]), f(inputs["bp"])

    Wq_f = sq[:, None] * Wq
    Wk_f = sk[:, None] * Wk
    Wv_f = sv[:, None] * Wv

    g = wdw * sdw
    bqkv = np.concatenate([bq, bk, bv])
    zscale = g
    zbias = g * bqkv + bdw

    def tiles2(a):   # (256, cols) -> [(128, cols)] * 2
        return [a[:128], a[128:]]

    wbig = np.concatenate(
        tiles2(Wq_f.T) + tiles2(Wk_f.T) + tiles2(Wv_f.T)
        + [(((spw[:, None] * Wpw) / 6.0).T)[128 * k:128 * (k + 1)]
           for k in range(4)]
        + tiles2((sp[:, None] * Wp).T)
        + [np.eye(128, dtype=np.float32)], axis=1)
    consts = {"wbig": f(wbig)}
    # padded head layout for the attention q/k weights (1/mean fold included)
    assert h == w, "mean folds assume H == W"
    wqTp = np.zeros((DIM, 256), np.float32)
    wkTp = np.zeros((DIM, 256), np.float32)
    qr_b = np.zeros((32, HEADS * h), np.float32)
    kr_b = np.zeros((32, HEADS * h), np.float32)
    qc_b = np.zeros((32, HEADS * w), np.float32)
    kc_b = np.zeros((32, HEADS * w), np.float32)
    pe_rq = _interp_pos_np(f(inputs["pe_rq"]), h)
    pe_rk = _interp_pos_np(f(inputs["pe_rk"]), h)
    pe_cq = _interp_pos_np(f(inputs["pe_cq"]), w)
    pe_ck = _interp_pos_np(f(inputs["pe_ck"]), w)
    for hh in range(HEADS):
        sl_p = slice(32 * hh, 32 * hh + KEY_DIM)
        sl_c = slice(KEY_DIM * hh, KEY_DIM * (hh + 1))
        wqTp[:, sl_p] = (Wq_f[sl_c, :] / w).T
        wkTp[:, sl_p] = (Wk_f[sl_c, :] / w).T
        qr_b[:KEY_DIM, h * hh:h * (hh + 1)] = bq[sl_c, None] + pe_rq[sl_c, :]
        kr_b[:KEY_DIM, h * hh:h * (hh + 1)] = bk[sl_c, None] + pe_rk[sl_c, :]
        qc_b[:KEY_DIM, w * hh:w * (hh + 1)] = bq[sl_c, None] + pe_cq[sl_c, :]
        kc_b[:KEY_DIM, w * hh:w * (hh + 1)] = bk[sl_c, None] + pe_ck[sl_c, :]
    id64pad = np.zeros((128, 64), np.float32)
    id64pad[:64] = np.eye(64, dtype=np.float32)
    wbigb = np.concatenate(
        tiles2((sr[:, None] * Wr).T) + tiles2((sc[:, None] * Wc).T)
        + tiles2(wqTp) + tiles2(wkTp) + tiles2(Wv_f.T)
        + [id64pad], axis=1)
    consts["wbigb"] = fb(wbigb)
    consts["qkbias"] = f(np.concatenate([qr_b, kr_b, qc_b, kc_b], axis=1))

    params = np.zeros((128, 20), np.float32)
    params[:, 0:4] = zscale.reshape(4, 128).T
    params[:, 4:8] = zbias.reshape(4, 128).T
    params[:, 8:10] = bv.reshape(2, 128).T
    params[:, 10:12] = (br + bv).reshape(2, 128).T
    params[:, 12:14] = bc.reshape(2, 128).T
    params[:, 14:16] = (bp + 3.0).reshape(2, 128).T
    params[:, 16:18] = (bpw / 6.0).reshape(2, 128).T
    consts["params"] = f(params)
    consts["onesW"] = np.full((max(h, w), 1), float(w),
                              ml_dtypes.bfloat16)
    consts["ones1"] = np.ones((1, 128), ml_dtypes.bfloat16)
    return consts


_NC_CACHE = {}


def _get_nc():
    if "nc" not in _NC_CACHE:
        _NC_CACHE["nc"] = build_nc()
    return _NC_CACHE["nc"]


def kernel(**inputs) -> np.ndarray:
    x = np.ascontiguousarray(np.asarray(inputs["x"], dtype=np.float32))
    consts = prepare_consts(inputs)
    nc = _get_nc()
    in_maps = []
    for c in range(N_CORES):
        m = dict(consts)
        m["x"] = np.ascontiguousarray(
            x[c * BPC:(c + 1) * BPC].reshape(BPC, DIM, H * W))
        in_maps.append(m)
    res = bass_utils.run_bass_kernel_spmd(nc, in_maps,
                                          core_ids=list(range(N_CORES)))
    y = np.concatenate([r["y"] for r in res.results], axis=0)
    return y.reshape(B, DIM, H, W)
